# revision 17
# baseline (speedup 1.0000x reference)
"""Expert-parallel MoE (top-2 routing) for 8 Trainium2 NeuronCores.

Strategy (hardcoded for the nn_MoE_28097676051036 problem shapes):
  - Host (numpy, float64): gating softmax, top-2 selection, gate
    normalization, aux loss, and token->expert dispatch (gather).
  - Device (one expert per core, SPMD over 8 cores): the two FFN matmuls
    h = relu(x @ w1 + b1), y_e = gate * (h @ w2), computed in bf16 with
    fp32 PSUM accumulation (DTYPE="f32r" gives ~2e-4 L2 error at ~25%
    more time; bf16 keeps the chip out of its DVFS power throttle).
  - Host: scatter-add per-expert outputs back into the token axis and add
    the (gate-weighted) b2 contribution.

Problem shapes: x[4,2048,512] f32, w_gate[8,512], w1[8,512,1024],
b1[8,1024], w2[8,1024,512], b2[8,512]; N=8192 tokens, top-2 of 8 experts.
"""

import numpy as np

K = 2
EPS = 1e-6
CVLOSS = 0.0
SWITCHLOSS = 0.01
ZLOSS = 0.001

B, L, D, H, E = 4, 2048, 512, 1024, 8
N = B * L
NCORES = 8
P = 128

# relu placement: "act" = ScalarE activation, "dve" = VectorE tensor_scalar
RELU_ENGINE = "act"
# matmul input dtype: "bf16" (~3e-3 L2 err, half the DMA bytes, full clocks)
# or "f32r" (reduced-precision fp32, ~2e-4 L2 err, ~25% slower end-to-end:
# the extra DMA+PE energy trips the chip-wide DVFS throttle)
DTYPE = "bf16"

_RUNNER_CACHE: dict = {}


def _build_bass(C: int, fold_gate: bool = True, dtype: str = "f32r"):
    """Build the per-core Bass module: FFN for one expert over C (padded)
    dispatched tokens. Inputs are laid out for the PE array:
      xt [D, C]  : gathered tokens, transposed; pre-scaled by the gate
                   when fold_gate (valid because gates > 0 and b1 == 0)
      w1 [D, H], w2 [H, D], b1 [128, H/128] (column per h-tile)
      g  [C, 1]  : only when not fold_gate
    Output y [C, D] = relu(xt.T @ w1 + b1) @ w2 (times g when not folded;
    b2 is added on host).
    """
    import concourse.mybir as mybir
    import concourse.tile as tile
    from concourse import bacc

    dtr = mybir.dt.float32r if dtype == "f32r" else mybir.dt.bfloat16
    dtf = mybir.dt.float32

    KD = D // P  # 4 k-tiles for the first matmul
    KH = H // P  # 8 k-tiles for the second matmul
    CT = C // P  # token tiles

    nc = bacc.Bacc()
    xt = nc.dram_tensor("xt", [D, C], dtr, kind="ExternalInput")
    w1 = nc.dram_tensor("w1", [D, H], dtr, kind="ExternalInput")
    b1 = nc.dram_tensor("b1", [P, KH], dtf, kind="ExternalInput")
    w2 = nc.dram_tensor("w2", [H, D], dtr, kind="ExternalInput")
    if not fold_gate:
        g = nc.dram_tensor("g", [C, 1], dtf, kind="ExternalInput")
        g_t = g.rearrange("(t p) o -> t p o", p=P)
    y = nc.dram_tensor("y", [C, D], dtf, kind="ExternalOutput")

    xt_t = xt.rearrange("(t p) c -> t p c", p=P)  # [KD, 128, C]
    w1_t = w1.rearrange("(t p) h -> t p h", p=P)  # [KD, 128, H]
    w2_t = w2.rearrange("(t p) d -> t p d", p=P)  # [KH, 128, D]
    y_t = y.rearrange("(t p) d -> t p d", p=P)  # [CT, 128, D]

    # free-dim chunks for stage A (moving operand max 512 for 4-byte dtypes;
    # f32r needs >=256 free dim for full rate)
    chunks = []
    c0 = 0
    while c0 < C:
        w = min(512, C - c0)
        chunks.append((c0, w))
        c0 += w

    with tile.TileContext(nc) as tc:
        with (
            tc.tile_pool(name="weights", bufs=1) as wpool,
            tc.tile_pool(name="data", bufs=1) as dpool,
            tc.tile_pool(name="out", bufs=4) as opool,
            tc.tile_pool(name="ps", bufs=8, space="PSUM") as psp,
        ):
            w1_sb = [wpool.tile([P, H], dtr, name=f"w1_{i}") for i in range(KD)]
            w2_sb = [wpool.tile([P, D], dtr, name=f"w2_{i}") for i in range(KH)]
            b1_sb = wpool.tile([P, KH], dtf, name="b1")
            xt_sb = [dpool.tile([P, C], dtr, name=f"xt_{i}") for i in range(KD)]
            if not fold_gate:
                g_sb = [dpool.tile([P, 1], dtf, name=f"g_{i}") for i in range(CT)]
            hT_sb = [dpool.tile([P, C], dtr, name=f"hT_{i}") for i in range(KH)]

            # PE warmup: dummy matmuls on a zeroed tile keep TensorE busy
            # from early in the kernel so the HAM clock-gate opens (K=8/8)
            # before the real matmuls start; overlaps the initial DMA wait.
            warm = dpool.tile([P, 512], mybir.dt.bfloat16, name="warm")
            nc.gpsimd.memset(warm[:], 0.0)
            for _ in range(16):
                wps = psp.tile([P, 512], mybir.dt.float32, name="ps")
                nc.tensor.matmul(
                    wps[:, :256], warm[:, 0:P], warm[:, 0:256], start=True, stop=True
                )

            # DMA order = consumption order. Each DMA instruction occupies its
            # issuing engine's HWDGE ring for ~650ns, so the critical first
            # loads are split across BOTH rings (sync + scalar) to halve the
            # serial issue latency. xt is split at column 1024 so stage A's
            # first two chunks don't wait for the whole tile. Output stores
            # ride the scalar ring (they start late).
            XSPLIT = min(1024, C)
            for i in range(KD):
                eng = nc.sync if i % 2 == 0 else nc.scalar
                eng.dma_start(out=w1_sb[i][:], in_=w1_t[i])
                eng.dma_start(out=xt_sb[i][:, 0:XSPLIT], in_=xt_t[i][:, 0:XSPLIT])
            nc.scalar.dma_start(out=b1_sb[:], in_=b1[:])
            if XSPLIT < C:
                for i in range(KD):
                    eng = nc.sync if i % 2 == 0 else nc.scalar
                    eng.dma_start(out=xt_sb[i][:, XSPLIT:], in_=xt_t[i][:, XSPLIT:])
            for i in range(KH):
                eng = nc.sync if i % 2 == 0 else nc.scalar
                eng.dma_start(out=w2_sb[i][:], in_=w2_t[i])
            if not fold_gate:
                for i in range(CT):
                    nc.sync.dma_start(out=g_sb[i][:], in_=g_t[i])

            # Stage A: hT[h-tile] = relu(w1.T-slice @ xg + b1), laid out
            # [h partitions, token free dim]. Chunk 0 runs kd-outer so each
            # arriving (w1_kd, xt_kd) pair immediately feeds 8 matmuls;
            # later chunks run h-outer with data already resident.
            ps_c0 = [psp.tile([P, 512], mybir.dt.float32, name="ps") for _ in range(KH)]
            c0, cw = chunks[0]
            for kd in range(KD):
                for h in range(KH):
                    nc.tensor.matmul(
                        ps_c0[h][:, :cw],
                        w1_sb[kd][:, h * P : (h + 1) * P],
                        xt_sb[kd][:, c0 : c0 + cw],
                        start=(kd == 0),
                        stop=(kd == KD - 1),
                    )
            def relu(dst, src, h):
                # In the fold_gate path b1 is guaranteed zero, so relu is a
                # plain max(x, 0) on the (otherwise idle) vector engine —
                # keeping ScalarE free for DMA issue. Otherwise ScalarE
                # activation applies the per-partition bias.
                if fold_gate:
                    nc.vector.tensor_scalar_max(dst, src, 0.0)
                else:
                    nc.scalar.activation(
                        dst,
                        src,
                        mybir.ActivationFunctionType.Relu,
                        bias=b1_sb[:, h : h + 1],
                    )

            for h in range(KH):
                relu(hT_sb[h][:, c0 : c0 + cw], ps_c0[h][:, :cw], h)
            for c0, cw in chunks[1:]:
                for h in range(KH):
                    ps = psp.tile([P, 512], mybir.dt.float32, name="ps")
                    for kd in range(KD):
                        nc.tensor.matmul(
                            ps[:, :cw],
                            w1_sb[kd][:, h * P : (h + 1) * P],
                            xt_sb[kd][:, c0 : c0 + cw],
                            start=(kd == 0),
                            stop=(kd == KD - 1),
                        )
                    relu(hT_sb[h][:, c0 : c0 + cw], ps[:, :cw], h)

            # Stage B: y[c-tile] = hT-slice.T @ w2 (times gate if not folded)
            for ct in range(CT):
                ps2 = psp.tile([P, D], mybir.dt.float32, name="ps")
                for h in range(KH):
                    nc.tensor.matmul(
                        ps2[:],
                        hT_sb[h][:, ct * P : (ct + 1) * P],
                        w2_sb[h][:],
                        start=(h == 0),
                        stop=(h == KH - 1),
                    )
                yt = opool.tile([P, D], dtf, name="yt")
                if fold_gate:
                    nc.scalar.copy(yt[:], ps2[:])
                else:
                    nc.vector.tensor_scalar_mul(yt[:], ps2[:], g_sb[ct][:, 0:1])
                nc.scalar.dma_start(out=y_t[ct], in_=yt[:])

    nc.finalize()
    return nc


def _make_runner(C: int, fold_gate: bool = True, dtype: str = "f32r"):
    """Compile the Bass module once and return a callable
    run(per_core_in_maps) -> list of per-core output dicts.
    Mirrors concourse.bass2jax.run_bass_via_pjrt but caches the jitted
    executable across invocations."""
    import jax
    import concourse.mybir as mybir
    from concourse import bass2jax
    from jax.experimental.shard_map import shard_map
    from jax.sharding import Mesh, PartitionSpec

    nc = _build_bass(C, fold_gate, dtype)
    bass2jax.install_neuronx_cc_hook()

    partition_name = nc.partition_id_tensor.name if nc.partition_id_tensor else None

    in_names = []
    out_names = []
    out_avals = []
    out_shapes = []
    for alloc in nc.m.functions[0].allocations:
        if not isinstance(alloc, mybir.MemoryLocationSet):
            continue
        name = alloc.memorylocations[0].name
        if alloc.kind == "ExternalInput":
            if name != partition_name:
                in_names.append(name)
        elif alloc.kind == "ExternalOutput":
            shape = tuple(alloc.tensor_shape)
            dtype = mybir.dt.np(alloc.dtype)
            out_names.append(name)
            out_avals.append(jax.core.ShapedArray(shape, dtype))
            out_shapes.append((shape, dtype))
    n_params = len(in_names)
    n_outs = len(out_names)
    all_in_names = in_names + out_names
    if partition_name is not None:
        all_in_names = all_in_names + [partition_name]

    donate = tuple(range(n_params, n_params + n_outs))

    def _body(*args):
        operands = list(args)
        if partition_name is not None:
            operands.append(bass2jax.partition_id_tensor())
        outs = bass2jax._bass_exec_p.bind(
            *operands,
            out_avals=tuple(out_avals),
            in_names=tuple(all_in_names),
            out_names=tuple(out_names),
            lowering_input_output_aliases=(),
            sim_require_finite=True,
            sim_require_nnan=True,
            nc=nc,
        )
        return tuple(outs)

    devices = jax.devices()[:NCORES]
    mesh = Mesh(np.asarray(devices), ("core",))
    in_specs = (PartitionSpec("core"),) * (n_params + n_outs)
    out_specs = (PartitionSpec("core"),) * n_outs
    sharded = jax.jit(
        shard_map(
            _body, mesh=mesh, in_specs=in_specs, out_specs=out_specs, check_rep=False
        ),
        donate_argnums=donate,
        keep_unused=True,
    )

    def run(per_core_in_maps):
        concat_in = [
            np.concatenate([m[name] for m in per_core_in_maps], axis=0)
            for name in in_names
        ]
        concat_zeros = [
            np.zeros((NCORES * s[0], *s[1:]), dt) for (s, dt) in out_shapes
        ]
        out_arrs = sharded(*concat_in, *concat_zeros)
        return [
            {
                name: np.asarray(out_arrs[i]).reshape(
                    NCORES, *out_shapes[i][0]
                )[c]
                for i, name in enumerate(out_names)
            }
            for c in range(NCORES)
        ]

    return run


def _get_runner(C: int, fold_gate: bool = True, dtype: str = "f32r"):
    key = (C, fold_gate, dtype)
    if key not in _RUNNER_CACHE:
        _RUNNER_CACHE[key] = _make_runner(C, fold_gate, dtype)
    return _RUNNER_CACHE[key]


def _routing(xf, w_gate):
    """Host-side gating in float64: returns top-2 indices [N,2], normalized
    gates [N,2] (f32), and the aux loss (f64 scalar)."""
    logits = xf.astype(np.float64) @ w_gate.astype(np.float64).T  # [N, E]
    m = logits.max(axis=1, keepdims=True)
    ex = np.exp(logits - m)
    sex = ex.sum(axis=1, keepdims=True)
    probs = ex / sex  # [N, E]

    idx = np.argsort(-probs, axis=1, kind="stable")[:, :K]  # [N, 2]
    rows = np.arange(N)[:, None]
    topg = probs[rows, idx]  # [N, 2]
    denom = topg.sum(axis=1, keepdims=True) + EPS
    gates = topg / denom  # [N, 2]

    # aux loss (CVLOSS term is multiplied by 0.0 in the reference)
    psum = probs.sum(axis=0)
    psum_n = psum / psum.sum()
    freqs = np.bincount(idx.ravel(), minlength=E).astype(np.float64)
    freqs_n = freqs / freqs.sum()
    switch = (psum_n * freqs_n).sum() * E
    lse = np.log(sex[:, 0]) + m[:, 0]
    zl = np.mean(lse**2)
    loss = SWITCHLOSS * switch + ZLOSS * zl
    if CVLOSS != 0.0:
        cv = np.var(psum_n, ddof=1) / (np.mean(psum_n) ** 2 + 1e-10)
        loss += CVLOSS * cv
    return idx, gates.astype(np.float32), loss


def _prepare(x, w_gate, w1, b1, w2):
    """Host-side routing + dispatch. Returns (in_maps, toks, C, fold_gate,
    idx, gates, loss)."""
    xf = x.reshape(N, D)
    idx, gates, loss = _routing(xf, w_gate)

    # dispatch: group (token, gate) pairs by expert
    flat_e = idx.ravel()
    flat_tok = np.repeat(np.arange(N), K)
    flat_g = gates.ravel()
    order = np.argsort(flat_e, kind="stable")
    counts = np.bincount(flat_e, minlength=E)
    offs = np.concatenate([[0], np.cumsum(counts)])

    C = int(np.ceil(counts.max() / P) * P)
    if C % 512 == P:  # avoid a 128-wide tail chunk (f32r slow below 256)
        C += P

    # Fold the gate into x (g * relu(z) == relu(g * z) since g > 0) —
    # only exact when b1 == 0, which setup_inputs guarantees.
    fold_gate = not np.any(b1)

    toks = []
    in_maps = []
    for e in range(E):
        sel = order[offs[e] : offs[e + 1]]
        te = flat_tok[sel]
        ge = flat_g[sel]
        toks.append(te)
        cnt = len(te)
        xg_T = np.zeros((D, C), dtype=np.float32)
        if fold_gate:
            xg_T[:, :cnt] = (xf[te] * ge[:, None]).T
        else:
            xg_T[:, :cnt] = xf[te].T
        if DTYPE == "bf16":
            import ml_dtypes

            bf = ml_dtypes.bfloat16
            m = {
                "xt": xg_T.astype(bf),
                "w1": np.ascontiguousarray(w1[e]).astype(bf),
                "b1": np.ascontiguousarray(b1[e].reshape(H // P, P).T),
                "w2": np.ascontiguousarray(w2[e]).astype(bf),
            }
        else:
            m = {
                "xt": xg_T,
                "w1": np.ascontiguousarray(w1[e]),
                "b1": np.ascontiguousarray(b1[e].reshape(H // P, P).T),
                "w2": np.ascontiguousarray(w2[e]),
            }
        if not fold_gate:
            g_arr = np.zeros((C, 1), dtype=np.float32)
            g_arr[:cnt, 0] = ge
            m["g"] = g_arr
        in_maps.append(m)
    return in_maps, toks, C, fold_gate, idx, gates, loss


def kernel(x, w_gate, w1, b1, w2, b2):
    x = np.asarray(x, dtype=np.float32)
    w_gate = np.asarray(w_gate, dtype=np.float32)
    w1 = np.asarray(w1, dtype=np.float32)
    b1 = np.asarray(b1, dtype=np.float32)
    w2 = np.asarray(w2, dtype=np.float32)
    b2 = np.asarray(b2, dtype=np.float32)

    in_maps, toks, C, fold_gate, idx, gates, loss = _prepare(x, w_gate, w1, b1, w2)

    run = _get_runner(C, fold_gate, DTYPE)
    outs = run(in_maps)

    y = np.zeros((N, D), dtype=np.float32)
    for e in range(E):
        cnt = len(toks[e])
        y[toks[e]] += outs[e]["y"][:cnt]

    # b2 contribution: y += sum_e gates_dense[:, e] * b2[e]
    gates_dense = np.zeros((N, E), dtype=np.float32)
    gates_dense[np.arange(N)[:, None], idx] = gates
    y += gates_dense @ b2

    return y.reshape(B, L, D), np.float32(loss)


# revision 20
# speedup vs baseline: 1.0425x; 1.0425x over previous
"""Expert-parallel MoE (top-2 routing) for 8 Trainium2 NeuronCores.

Strategy (hardcoded for the nn_MoE_28097676051036 problem shapes):
  - Host (numpy, float64): gating softmax, top-2 selection, gate
    normalization, aux loss, and token->expert dispatch (gather).
  - Device (one expert per core, SPMD over 8 cores): the two FFN matmuls
    h = relu(x @ w1 + b1), y_e = gate * (h @ w2), computed in bf16 with
    fp32 PSUM accumulation (DTYPE="f32r" gives ~2e-4 L2 error at ~25%
    more time; bf16 keeps the chip out of its DVFS power throttle).
  - Host: scatter-add per-expert outputs back into the token axis and add
    the (gate-weighted) b2 contribution.

Problem shapes: x[4,2048,512] f32, w_gate[8,512], w1[8,512,1024],
b1[8,1024], w2[8,1024,512], b2[8,512]; N=8192 tokens, top-2 of 8 experts.
"""

import numpy as np

K = 2
EPS = 1e-6
CVLOSS = 0.0
SWITCHLOSS = 0.01
ZLOSS = 0.001

B, L, D, H, E = 4, 2048, 512, 1024, 8
N = B * L
NCORES = 8
P = 128

# relu placement: "act" = ScalarE activation, "dve" = VectorE tensor_scalar
RELU_ENGINE = "act"
# matmul input dtype: "bf16" (~3e-3 L2 err, half the DMA bytes, full clocks)
# or "f32r" (reduced-precision fp32, ~2e-4 L2 err, ~25% slower end-to-end:
# the extra DMA+PE energy trips the chip-wide DVFS throttle)
DTYPE = "bf16"

_RUNNER_CACHE: dict = {}


def _build_bass(C: int, fold_gate: bool = True, dtype: str = "f32r"):
    """Build the per-core Bass module: FFN for one expert over C (padded)
    dispatched tokens. Inputs are laid out for the PE array:
      xt [D, C]  : gathered tokens, transposed; pre-scaled by the gate
                   when fold_gate (valid because gates > 0 and b1 == 0)
      w1 [D, H], w2 [H, D], b1 [128, H/128] (column per h-tile)
      g  [C, 1]  : only when not fold_gate
    Output y [C, D] = relu(xt.T @ w1 + b1) @ w2 (times g when not folded;
    b2 is added on host).
    """
    import concourse.mybir as mybir
    import concourse.tile as tile
    from concourse import bacc

    dtr = mybir.dt.float32r if dtype == "f32r" else mybir.dt.bfloat16
    dtf = mybir.dt.float32

    KD = D // P  # 4 k-tiles for the first matmul
    KH = H // P  # 8 k-tiles for the second matmul
    CT = C // P  # token tiles

    nc = bacc.Bacc()
    xt = nc.dram_tensor("xt", [D, C], dtr, kind="ExternalInput")
    w1 = nc.dram_tensor("w1", [D, H], dtr, kind="ExternalInput")
    b1 = nc.dram_tensor("b1", [P, KH], dtf, kind="ExternalInput")
    w2 = nc.dram_tensor("w2", [H, D], dtr, kind="ExternalInput")
    if not fold_gate:
        g = nc.dram_tensor("g", [C, 1], dtf, kind="ExternalInput")
        g_t = g.rearrange("(t p) o -> t p o", p=P)
    y = nc.dram_tensor("y", [C, D], dtf, kind="ExternalOutput")

    xt_t = xt.rearrange("(t p) c -> t p c", p=P)  # [KD, 128, C]
    w1_t = w1.rearrange("(t p) h -> t p h", p=P)  # [KD, 128, H]
    w2_t = w2.rearrange("(t p) d -> t p d", p=P)  # [KH, 128, D]
    y_t = y.rearrange("(t p) d -> t p d", p=P)  # [CT, 128, D]

    # free-dim chunks for stage A (moving operand max 512 for 4-byte dtypes;
    # f32r needs >=256 free dim for full rate)
    chunks = []
    c0 = 0
    while c0 < C:
        w = min(512, C - c0)
        chunks.append((c0, w))
        c0 += w

    with tile.TileContext(nc) as tc:
        with (
            tc.tile_pool(name="weights", bufs=1) as wpool,
            tc.tile_pool(name="data", bufs=1) as dpool,
            tc.tile_pool(name="out", bufs=4) as opool,
            tc.tile_pool(name="ps", bufs=8, space="PSUM") as psp,
        ):
            w1_sb = [wpool.tile([P, H], dtr, name=f"w1_{i}") for i in range(KD)]
            w2_sb = [wpool.tile([P, D], dtr, name=f"w2_{i}") for i in range(KH)]
            b1_sb = wpool.tile([P, KH], dtf, name="b1")
            xt_sb = [dpool.tile([P, C], dtr, name=f"xt_{i}") for i in range(KD)]
            if not fold_gate:
                g_sb = [dpool.tile([P, 1], dtf, name=f"g_{i}") for i in range(CT)]
            hT_sb = [dpool.tile([P, C], dtr, name=f"hT_{i}") for i in range(KH)]

            # PE warmup: dummy matmuls on a zeroed tile keep TensorE busy
            # from early in the kernel so the HAM clock-gate opens (K=8/8)
            # before the real matmuls start; overlaps the initial DMA wait.
            warm = dpool.tile([P, 512], mybir.dt.bfloat16, name="warm")
            nc.gpsimd.memset(warm[:], 0.0)
            for _ in range(16):
                wps = psp.tile([P, 512], mybir.dt.float32, name="ps")
                nc.tensor.matmul(
                    wps[:, :256], warm[:, 0:P], warm[:, 0:256], start=True, stop=True
                )

            # DMA order = consumption order. Each DMA instruction occupies its
            # issuing engine's HWDGE ring for ~650ns, so the critical first
            # loads are split across BOTH rings (sync + scalar) to halve the
            # serial issue latency. xt is split at column 1024 so stage A's
            # first two chunks don't wait for the whole tile. Output stores
            # ride the scalar ring (they start late).
            XSPLIT = min(1024, C)
            for i in range(KD):
                eng = nc.sync if i % 2 == 0 else nc.scalar
                eng.dma_start(out=w1_sb[i][:], in_=w1_t[i])
                eng.dma_start(out=xt_sb[i][:, 0:XSPLIT], in_=xt_t[i][:, 0:XSPLIT])
            nc.scalar.dma_start(out=b1_sb[:], in_=b1[:])
            if XSPLIT < C:
                for i in range(KD):
                    eng = nc.sync if i % 2 == 0 else nc.scalar
                    eng.dma_start(out=xt_sb[i][:, XSPLIT:], in_=xt_t[i][:, XSPLIT:])
            for i in range(KH):
                eng = nc.sync if i % 2 == 0 else nc.scalar
                eng.dma_start(out=w2_sb[i][:], in_=w2_t[i])
            if not fold_gate:
                for i in range(CT):
                    nc.sync.dma_start(out=g_sb[i][:], in_=g_t[i])

            # Stage A: hT[h-tile] = relu(w1.T-slice @ xg + b1), laid out
            # [h partitions, token free dim]. Chunk 0 runs kd-outer so each
            # arriving (w1_kd, xt_kd) pair immediately feeds 8 matmuls;
            # later chunks run h-outer with data already resident.
            ps_c0 = [psp.tile([P, 512], mybir.dt.float32, name="ps") for _ in range(KH)]
            c0, cw = chunks[0]
            for kd in range(KD):
                for h in range(KH):
                    nc.tensor.matmul(
                        ps_c0[h][:, :cw],
                        w1_sb[kd][:, h * P : (h + 1) * P],
                        xt_sb[kd][:, c0 : c0 + cw],
                        start=(kd == 0),
                        stop=(kd == KD - 1),
                    )
            def relu(dst, src, h):
                # In the fold_gate path b1 is guaranteed zero, so relu is a
                # plain max(x, 0) on the (otherwise idle) vector engine —
                # keeping ScalarE free for DMA issue. Otherwise ScalarE
                # activation applies the per-partition bias.
                if fold_gate:
                    nc.vector.tensor_scalar_max(dst, src, 0.0)
                else:
                    nc.scalar.activation(
                        dst,
                        src,
                        mybir.ActivationFunctionType.Relu,
                        bias=b1_sb[:, h : h + 1],
                    )

            for h in range(KH):
                relu(hT_sb[h][:, c0 : c0 + cw], ps_c0[h][:, :cw], h)
            for c0, cw in chunks[1:]:
                for h in range(KH):
                    ps = psp.tile([P, 512], mybir.dt.float32, name="ps")
                    for kd in range(KD):
                        nc.tensor.matmul(
                            ps[:, :cw],
                            w1_sb[kd][:, h * P : (h + 1) * P],
                            xt_sb[kd][:, c0 : c0 + cw],
                            start=(kd == 0),
                            stop=(kd == KD - 1),
                        )
                    relu(hT_sb[h][:, c0 : c0 + cw], ps[:, :cw], h)

            # Stage B: y[c-tile] = hT-slice.T @ w2 (times gate if not folded)
            for ct in range(CT):
                ps2 = psp.tile([P, D], mybir.dt.float32, name="ps")
                for h in range(KH):
                    nc.tensor.matmul(
                        ps2[:],
                        hT_sb[h][:, ct * P : (ct + 1) * P],
                        w2_sb[h][:],
                        start=(h == 0),
                        stop=(h == KH - 1),
                    )
                yt = opool.tile([P, D], dtf, name="yt")
                if fold_gate:
                    nc.scalar.copy(yt[:], ps2[:])
                else:
                    nc.vector.tensor_scalar_mul(yt[:], ps2[:], g_sb[ct][:, 0:1])
                nc.scalar.dma_start(out=y_t[ct], in_=yt[:])

    nc.finalize()
    return nc


def _make_runner(C: int, fold_gate: bool = True, dtype: str = "f32r"):
    """Compile the Bass module once and return a callable
    run(per_core_in_maps) -> list of per-core output dicts.
    Mirrors concourse.bass2jax.run_bass_via_pjrt but caches the jitted
    executable across invocations."""
    import jax
    import concourse.mybir as mybir
    from concourse import bass2jax
    from jax.experimental.shard_map import shard_map
    from jax.sharding import Mesh, PartitionSpec

    nc = _build_bass(C, fold_gate, dtype)
    bass2jax.install_neuronx_cc_hook()

    partition_name = nc.partition_id_tensor.name if nc.partition_id_tensor else None

    in_names = []
    out_names = []
    out_avals = []
    out_shapes = []
    for alloc in nc.m.functions[0].allocations:
        if not isinstance(alloc, mybir.MemoryLocationSet):
            continue
        name = alloc.memorylocations[0].name
        if alloc.kind == "ExternalInput":
            if name != partition_name:
                in_names.append(name)
        elif alloc.kind == "ExternalOutput":
            shape = tuple(alloc.tensor_shape)
            dtype = mybir.dt.np(alloc.dtype)
            out_names.append(name)
            out_avals.append(jax.core.ShapedArray(shape, dtype))
            out_shapes.append((shape, dtype))
    n_params = len(in_names)
    n_outs = len(out_names)
    all_in_names = in_names + out_names
    if partition_name is not None:
        all_in_names = all_in_names + [partition_name]

    donate = tuple(range(n_params, n_params + n_outs))

    def _body(*args):
        operands = list(args)
        if partition_name is not None:
            operands.append(bass2jax.partition_id_tensor())
        outs = bass2jax._bass_exec_p.bind(
            *operands,
            out_avals=tuple(out_avals),
            in_names=tuple(all_in_names),
            out_names=tuple(out_names),
            lowering_input_output_aliases=(),
            sim_require_finite=True,
            sim_require_nnan=True,
            nc=nc,
        )
        return tuple(outs)

    devices = jax.devices()[:NCORES]
    mesh = Mesh(np.asarray(devices), ("core",))
    in_specs = (PartitionSpec("core"),) * (n_params + n_outs)
    out_specs = (PartitionSpec("core"),) * n_outs
    sharded = jax.jit(
        shard_map(
            _body, mesh=mesh, in_specs=in_specs, out_specs=out_specs, check_rep=False
        ),
        donate_argnums=donate,
        keep_unused=True,
    )

    def run(per_core_in_maps):
        concat_in = [
            np.concatenate([m[name] for m in per_core_in_maps], axis=0)
            for name in in_names
        ]
        concat_zeros = [
            np.zeros((NCORES * s[0], *s[1:]), dt) for (s, dt) in out_shapes
        ]
        out_arrs = sharded(*concat_in, *concat_zeros)
        return [
            {
                name: np.asarray(out_arrs[i]).reshape(
                    NCORES, *out_shapes[i][0]
                )[c]
                for i, name in enumerate(out_names)
            }
            for c in range(NCORES)
        ]

    return run


def _get_runner(C: int, fold_gate: bool = True, dtype: str = "f32r"):
    key = (C, fold_gate, dtype)
    if key not in _RUNNER_CACHE:
        _RUNNER_CACHE[key] = _make_runner(C, fold_gate, dtype)
    return _RUNNER_CACHE[key]


def _routing(xf, w_gate):
    """Host-side gating in float64: returns top-2 indices [N,2], normalized
    gates [N,2] (f32), and the aux loss (f64 scalar)."""
    logits = xf.astype(np.float64) @ w_gate.astype(np.float64).T  # [N, E]
    m = logits.max(axis=1, keepdims=True)
    ex = np.exp(logits - m)
    sex = ex.sum(axis=1, keepdims=True)
    probs = ex / sex  # [N, E]

    idx = np.argsort(-probs, axis=1, kind="stable")[:, :K]  # [N, 2]
    rows = np.arange(N)[:, None]
    topg = probs[rows, idx]  # [N, 2]
    denom = topg.sum(axis=1, keepdims=True) + EPS
    gates = topg / denom  # [N, 2]

    # aux loss (CVLOSS term is multiplied by 0.0 in the reference)
    psum = probs.sum(axis=0)
    psum_n = psum / psum.sum()
    freqs = np.bincount(idx.ravel(), minlength=E).astype(np.float64)
    freqs_n = freqs / freqs.sum()
    switch = (psum_n * freqs_n).sum() * E
    lse = np.log(sex[:, 0]) + m[:, 0]
    zl = np.mean(lse**2)
    loss = SWITCHLOSS * switch + ZLOSS * zl
    if CVLOSS != 0.0:
        cv = np.var(psum_n, ddof=1) / (np.mean(psum_n) ** 2 + 1e-10)
        loss += CVLOSS * cv
    return idx, gates.astype(np.float32), loss


# Per-core token capacity (capacity factor 1.0: N*K/NCORES). Tokens routed
# to an expert beyond its capacity are the usual MoE "overflow"; they get an
# exact f32 FFN on the host (~2% of pairs for these inputs) instead of
# padding every core's matmul stream up to the most-loaded expert.
CAPACITY = 2048


def _prepare(x, w_gate, w1, b1, w2):
    """Host-side routing + dispatch. Returns (in_maps, toks, C, fold_gate,
    idx, gates, loss, overflow) where overflow is [(e, tok_idx, gate_vals)]."""
    xf = x.reshape(N, D)
    idx, gates, loss = _routing(xf, w_gate)

    # dispatch: group (token, gate) pairs by expert
    flat_e = idx.ravel()
    flat_tok = np.repeat(np.arange(N), K)
    flat_g = gates.ravel()
    order = np.argsort(flat_e, kind="stable")
    counts = np.bincount(flat_e, minlength=E)
    offs = np.concatenate([[0], np.cumsum(counts)])

    C = int(np.ceil(min(counts.max(), CAPACITY) / P) * P)
    if C % 512 == P and DTYPE == "f32r":
        C += P  # avoid a 128-wide tail chunk (f32r slow below 256 free dim)

    # Fold the gate into x (g * relu(z) == relu(g * z) since g > 0) —
    # only exact when b1 == 0, which setup_inputs guarantees.
    fold_gate = not np.any(b1)

    toks = []
    in_maps = []
    overflow = []
    for e in range(E):
        sel = order[offs[e] : offs[e + 1]]
        te = flat_tok[sel]
        ge = flat_g[sel]
        if len(te) > C:
            overflow.append((e, te[C:], ge[C:]))
            te = te[:C]
            ge = ge[:C]
        toks.append(te)
        cnt = len(te)
        xg_T = np.zeros((D, C), dtype=np.float32)
        if fold_gate:
            xg_T[:, :cnt] = (xf[te] * ge[:, None]).T
        else:
            xg_T[:, :cnt] = xf[te].T
        if DTYPE == "bf16":
            import ml_dtypes

            bf = ml_dtypes.bfloat16
            m = {
                "xt": xg_T.astype(bf),
                "w1": np.ascontiguousarray(w1[e]).astype(bf),
                "b1": np.ascontiguousarray(b1[e].reshape(H // P, P).T),
                "w2": np.ascontiguousarray(w2[e]).astype(bf),
            }
        else:
            m = {
                "xt": xg_T,
                "w1": np.ascontiguousarray(w1[e]),
                "b1": np.ascontiguousarray(b1[e].reshape(H // P, P).T),
                "w2": np.ascontiguousarray(w2[e]),
            }
        if not fold_gate:
            g_arr = np.zeros((C, 1), dtype=np.float32)
            g_arr[:cnt, 0] = ge
            m["g"] = g_arr
        in_maps.append(m)
    return in_maps, toks, C, fold_gate, idx, gates, loss, overflow


def kernel(x, w_gate, w1, b1, w2, b2):
    x = np.asarray(x, dtype=np.float32)
    w_gate = np.asarray(w_gate, dtype=np.float32)
    w1 = np.asarray(w1, dtype=np.float32)
    b1 = np.asarray(b1, dtype=np.float32)
    w2 = np.asarray(w2, dtype=np.float32)
    b2 = np.asarray(b2, dtype=np.float32)

    in_maps, toks, C, fold_gate, idx, gates, loss, overflow = _prepare(
        x, w_gate, w1, b1, w2
    )

    run = _get_runner(C, fold_gate, DTYPE)
    outs = run(in_maps)

    y = np.zeros((N, D), dtype=np.float32)
    for e in range(E):
        cnt = len(toks[e])
        y[toks[e]] += outs[e]["y"][:cnt]

    # capacity-overflow tokens: exact f32 FFN on host (b2 excluded here,
    # it is added for all tokens via gates_dense @ b2 below)
    for e, te, ge in overflow:
        xo = x.reshape(N, D)[te]
        ho = np.maximum(xo @ w1[e] + b1[e], 0.0)
        y[te] += ge[:, None] * (ho @ w2[e])

    # b2 contribution: y += sum_e gates_dense[:, e] * b2[e]
    gates_dense = np.zeros((N, E), dtype=np.float32)
    gates_dense[np.arange(N)[:, None], idx] = gates
    y += gates_dense @ b2

    return y.reshape(B, L, D), np.float32(loss)


# revision 22
# speedup vs baseline: 1.0892x; 1.0448x over previous
"""Expert-parallel MoE (top-2 routing) for 8 Trainium2 NeuronCores.

Strategy (hardcoded for the nn_MoE_28097676051036 problem shapes):
  - Host (numpy, float64): gating softmax, top-2 selection, gate
    normalization, aux loss, and token->expert dispatch (gather).
  - Device (one expert per core, SPMD over 8 cores): the two FFN matmuls
    h = relu(x @ w1 + b1), y_e = gate * (h @ w2), computed in bf16 with
    fp32 PSUM accumulation (DTYPE="f32r" gives ~2e-4 L2 error at ~25%
    more time; bf16 keeps the chip out of its DVFS power throttle).
  - Host: scatter-add per-expert outputs back into the token axis and add
    the (gate-weighted) b2 contribution.

Problem shapes: x[4,2048,512] f32, w_gate[8,512], w1[8,512,1024],
b1[8,1024], w2[8,1024,512], b2[8,512]; N=8192 tokens, top-2 of 8 experts.
"""

import numpy as np

K = 2
EPS = 1e-6
CVLOSS = 0.0
SWITCHLOSS = 0.01
ZLOSS = 0.001

B, L, D, H, E = 4, 2048, 512, 1024, 8
N = B * L
NCORES = 8
P = 128

# relu placement: "act" = ScalarE activation, "dve" = VectorE tensor_scalar
RELU_ENGINE = "act"
# matmul input dtype: "bf16" (~3e-3 L2 err, half the DMA bytes, full clocks)
# or "f32r" (reduced-precision fp32, ~2e-4 L2 err, ~25% slower end-to-end:
# the extra DMA+PE energy trips the chip-wide DVFS throttle)
DTYPE = "bf16"

_RUNNER_CACHE: dict = {}


def _build_bass(C: int, fold_gate: bool = True, dtype: str = "f32r"):
    """Build the per-core Bass module: FFN for one expert over C (padded)
    dispatched tokens. Inputs are laid out for the PE array:
      xt [D, C]  : gathered tokens, transposed; pre-scaled by the gate
                   when fold_gate (valid because gates > 0 and b1 == 0)
      w1 [D, H], w2 [H, D], b1 [128, H/128] (column per h-tile)
      g  [C, 1]  : only when not fold_gate
    Output y [C, D] = relu(xt.T @ w1 + b1) @ w2 (times g when not folded;
    b2 is added on host).
    """
    import concourse.mybir as mybir
    import concourse.tile as tile
    from concourse import bacc

    dtr = mybir.dt.float32r if dtype == "f32r" else mybir.dt.bfloat16
    dtf = mybir.dt.float32

    KD = D // P  # 4 k-tiles for the first matmul
    KH = H // P  # 8 k-tiles for the second matmul
    CT = C // P  # token tiles

    nc = bacc.Bacc()
    xt = nc.dram_tensor("xt", [D, C], dtr, kind="ExternalInput")
    w1 = nc.dram_tensor("w1", [D, H], dtr, kind="ExternalInput")
    if not fold_gate:
        # the folded path needs no bias (b1 == 0 is a precondition)
        b1 = nc.dram_tensor("b1", [P, KH], dtf, kind="ExternalInput")
    w2 = nc.dram_tensor("w2", [H, D], dtr, kind="ExternalInput")
    if not fold_gate:
        g = nc.dram_tensor("g", [C, 1], dtf, kind="ExternalInput")
        g_t = g.rearrange("(t p) o -> t p o", p=P)
    y = nc.dram_tensor("y", [C, D], dtf, kind="ExternalOutput")

    xt_t = xt.rearrange("(t p) c -> t p c", p=P)  # [KD, 128, C]
    w1_t = w1.rearrange("(t p) h -> t p h", p=P)  # [KD, 128, H]
    w2_t = w2.rearrange("(t p) d -> t p d", p=P)  # [KH, 128, D]
    y_t = y.rearrange("(t p) d -> t p d", p=P)  # [CT, 128, D]

    # free-dim chunks for stage A (moving operand max 512 for 4-byte dtypes;
    # f32r needs >=256 free dim for full rate)
    chunks = []
    c0 = 0
    while c0 < C:
        w = min(512, C - c0)
        chunks.append((c0, w))
        c0 += w

    with tile.TileContext(nc) as tc:
        with (
            tc.tile_pool(name="weights", bufs=1) as wpool,
            tc.tile_pool(name="data", bufs=1) as dpool,
            tc.tile_pool(name="out", bufs=4) as opool,
            tc.tile_pool(name="ps", bufs=8, space="PSUM") as psp,
        ):
            w1_sb = [wpool.tile([P, H], dtr, name=f"w1_{i}") for i in range(KD)]
            w2_sb = [wpool.tile([P, D], dtr, name=f"w2_{i}") for i in range(KH)]
            b1_sb = None if fold_gate else wpool.tile([P, KH], dtf, name="b1")
            xt_sb = [dpool.tile([P, C], dtr, name=f"xt_{i}") for i in range(KD)]
            if not fold_gate:
                g_sb = [dpool.tile([P, 1], dtf, name=f"g_{i}") for i in range(CT)]
            hT_sb = [dpool.tile([P, C], dtr, name=f"hT_{i}") for i in range(KH)]

            # PE warmup: dummy matmuls on a zeroed tile keep TensorE busy
            # from early in the kernel so the HAM clock-gate opens (K=8/8)
            # before the real matmuls start; overlaps the initial DMA wait.
            warm = dpool.tile([P, 512], mybir.dt.bfloat16, name="warm")
            nc.gpsimd.memset(warm[:], 0.0)
            for _ in range(14):
                wps = psp.tile([P, 512], mybir.dt.float32, name="ps")
                nc.tensor.matmul(
                    wps[:, :256], warm[:, 0:P], warm[:, 0:256], start=True, stop=True
                )

            # DMA order = consumption order. Each DMA instruction occupies its
            # issuing engine's HWDGE ring for ~650ns, so the critical first
            # loads are split across BOTH rings (sync + scalar) to halve the
            # serial issue latency. xt is split at column 1024 so stage A's
            # first two chunks don't wait for the whole tile. Output stores
            # ride the scalar ring (they start late).
            XSPLIT = min(512, C)
            for i in range(KD):
                eng = nc.sync if i % 2 == 0 else nc.scalar
                eng.dma_start(out=w1_sb[i][:], in_=w1_t[i])
                eng.dma_start(out=xt_sb[i][:, 0:XSPLIT], in_=xt_t[i][:, 0:XSPLIT])
            if not fold_gate:
                nc.scalar.dma_start(out=b1_sb[:], in_=b1[:])
            if XSPLIT < C:
                for i in range(KD):
                    eng = nc.sync if i % 2 == 0 else nc.scalar
                    eng.dma_start(out=xt_sb[i][:, XSPLIT:], in_=xt_t[i][:, XSPLIT:])
            for i in range(KH):
                eng = nc.sync if i % 2 == 0 else nc.scalar
                eng.dma_start(out=w2_sb[i][:], in_=w2_t[i])
            if not fold_gate:
                for i in range(CT):
                    nc.sync.dma_start(out=g_sb[i][:], in_=g_t[i])

            # Stage A: hT[h-tile] = relu(w1.T-slice @ xg + b1), laid out
            # [h partitions, token free dim]. Chunk 0 runs kd-outer so each
            # arriving (w1_kd, xt_kd) pair immediately feeds 8 matmuls;
            # later chunks run h-outer with data already resident.
            ps_c0 = [psp.tile([P, 512], mybir.dt.float32, name="ps") for _ in range(KH)]
            c0, cw = chunks[0]
            for kd in range(KD):
                for h in range(KH):
                    nc.tensor.matmul(
                        ps_c0[h][:, :cw],
                        w1_sb[kd][:, h * P : (h + 1) * P],
                        xt_sb[kd][:, c0 : c0 + cw],
                        start=(kd == 0),
                        stop=(kd == KD - 1),
                    )
            def relu(dst, src, h):
                # In the fold_gate path b1 is guaranteed zero, so relu is a
                # plain max(x, 0) on the (otherwise idle) vector engine —
                # keeping ScalarE free for DMA issue. Otherwise ScalarE
                # activation applies the per-partition bias.
                if fold_gate:
                    nc.vector.tensor_scalar_max(dst, src, 0.0)
                else:
                    nc.scalar.activation(
                        dst,
                        src,
                        mybir.ActivationFunctionType.Relu,
                        bias=b1_sb[:, h : h + 1],
                    )

            for h in range(KH):
                relu(hT_sb[h][:, c0 : c0 + cw], ps_c0[h][:, :cw], h)
            for c0, cw in chunks[1:]:
                for h in range(KH):
                    ps = psp.tile([P, 512], mybir.dt.float32, name="ps")
                    for kd in range(KD):
                        nc.tensor.matmul(
                            ps[:, :cw],
                            w1_sb[kd][:, h * P : (h + 1) * P],
                            xt_sb[kd][:, c0 : c0 + cw],
                            start=(kd == 0),
                            stop=(kd == KD - 1),
                        )
                    relu(hT_sb[h][:, c0 : c0 + cw], ps[:, :cw], h)

            # Stage B: y[c-tile] = hT-slice.T @ w2 (times gate if not folded)
            for ct in range(CT):
                ps2 = psp.tile([P, D], mybir.dt.float32, name="ps")
                for h in range(KH):
                    nc.tensor.matmul(
                        ps2[:],
                        hT_sb[h][:, ct * P : (ct + 1) * P],
                        w2_sb[h][:],
                        start=(h == 0),
                        stop=(h == KH - 1),
                    )
                yt = opool.tile([P, D], dtf, name="yt")
                if fold_gate:
                    nc.scalar.copy(yt[:], ps2[:])
                else:
                    nc.vector.tensor_scalar_mul(yt[:], ps2[:], g_sb[ct][:, 0:1])
                nc.scalar.dma_start(out=y_t[ct], in_=yt[:])

    nc.finalize()
    return nc


def _make_runner(C: int, fold_gate: bool = True, dtype: str = "f32r"):
    """Compile the Bass module once and return a callable
    run(per_core_in_maps) -> list of per-core output dicts.
    Mirrors concourse.bass2jax.run_bass_via_pjrt but caches the jitted
    executable across invocations."""
    import jax
    import concourse.mybir as mybir
    from concourse import bass2jax
    from jax.experimental.shard_map import shard_map
    from jax.sharding import Mesh, PartitionSpec

    nc = _build_bass(C, fold_gate, dtype)
    bass2jax.install_neuronx_cc_hook()

    partition_name = nc.partition_id_tensor.name if nc.partition_id_tensor else None

    in_names = []
    out_names = []
    out_avals = []
    out_shapes = []
    for alloc in nc.m.functions[0].allocations:
        if not isinstance(alloc, mybir.MemoryLocationSet):
            continue
        name = alloc.memorylocations[0].name
        if alloc.kind == "ExternalInput":
            if name != partition_name:
                in_names.append(name)
        elif alloc.kind == "ExternalOutput":
            shape = tuple(alloc.tensor_shape)
            dtype = mybir.dt.np(alloc.dtype)
            out_names.append(name)
            out_avals.append(jax.core.ShapedArray(shape, dtype))
            out_shapes.append((shape, dtype))
    n_params = len(in_names)
    n_outs = len(out_names)
    all_in_names = in_names + out_names
    if partition_name is not None:
        all_in_names = all_in_names + [partition_name]

    donate = tuple(range(n_params, n_params + n_outs))

    def _body(*args):
        operands = list(args)
        if partition_name is not None:
            operands.append(bass2jax.partition_id_tensor())
        outs = bass2jax._bass_exec_p.bind(
            *operands,
            out_avals=tuple(out_avals),
            in_names=tuple(all_in_names),
            out_names=tuple(out_names),
            lowering_input_output_aliases=(),
            sim_require_finite=True,
            sim_require_nnan=True,
            nc=nc,
        )
        return tuple(outs)

    devices = jax.devices()[:NCORES]
    mesh = Mesh(np.asarray(devices), ("core",))
    in_specs = (PartitionSpec("core"),) * (n_params + n_outs)
    out_specs = (PartitionSpec("core"),) * n_outs
    sharded = jax.jit(
        shard_map(
            _body, mesh=mesh, in_specs=in_specs, out_specs=out_specs, check_rep=False
        ),
        donate_argnums=donate,
        keep_unused=True,
    )

    def run(per_core_in_maps):
        concat_in = [
            np.concatenate([m[name] for m in per_core_in_maps], axis=0)
            for name in in_names
        ]
        concat_zeros = [
            np.zeros((NCORES * s[0], *s[1:]), dt) for (s, dt) in out_shapes
        ]
        out_arrs = sharded(*concat_in, *concat_zeros)
        return [
            {
                name: np.asarray(out_arrs[i]).reshape(
                    NCORES, *out_shapes[i][0]
                )[c]
                for i, name in enumerate(out_names)
            }
            for c in range(NCORES)
        ]

    return run


def _get_runner(C: int, fold_gate: bool = True, dtype: str = "f32r"):
    key = (C, fold_gate, dtype)
    if key not in _RUNNER_CACHE:
        _RUNNER_CACHE[key] = _make_runner(C, fold_gate, dtype)
    return _RUNNER_CACHE[key]


def _routing(xf, w_gate):
    """Host-side gating in float64: returns top-2 indices [N,2], normalized
    gates [N,2] (f32), and the aux loss (f64 scalar)."""
    logits = xf.astype(np.float64) @ w_gate.astype(np.float64).T  # [N, E]
    m = logits.max(axis=1, keepdims=True)
    ex = np.exp(logits - m)
    sex = ex.sum(axis=1, keepdims=True)
    probs = ex / sex  # [N, E]

    idx = np.argsort(-probs, axis=1, kind="stable")[:, :K]  # [N, 2]
    rows = np.arange(N)[:, None]
    topg = probs[rows, idx]  # [N, 2]
    denom = topg.sum(axis=1, keepdims=True) + EPS
    gates = topg / denom  # [N, 2]

    # aux loss (CVLOSS term is multiplied by 0.0 in the reference)
    psum = probs.sum(axis=0)
    psum_n = psum / psum.sum()
    freqs = np.bincount(idx.ravel(), minlength=E).astype(np.float64)
    freqs_n = freqs / freqs.sum()
    switch = (psum_n * freqs_n).sum() * E
    lse = np.log(sex[:, 0]) + m[:, 0]
    zl = np.mean(lse**2)
    loss = SWITCHLOSS * switch + ZLOSS * zl
    if CVLOSS != 0.0:
        cv = np.var(psum_n, ddof=1) / (np.mean(psum_n) ** 2 + 1e-10)
        loss += CVLOSS * cv
    return idx, gates.astype(np.float32), loss


# Per-core token capacity (capacity factor 1.0: N*K/NCORES). Tokens routed
# to an expert beyond its capacity are the usual MoE "overflow"; they get an
# exact f32 FFN on the host (~2% of pairs for these inputs) instead of
# padding every core's matmul stream up to the most-loaded expert.
CAPACITY = 2048


def _prepare(x, w_gate, w1, b1, w2):
    """Host-side routing + dispatch. Returns (in_maps, toks, C, fold_gate,
    idx, gates, loss, overflow) where overflow is [(e, tok_idx, gate_vals)]."""
    xf = x.reshape(N, D)
    idx, gates, loss = _routing(xf, w_gate)

    # dispatch: group (token, gate) pairs by expert
    flat_e = idx.ravel()
    flat_tok = np.repeat(np.arange(N), K)
    flat_g = gates.ravel()
    order = np.argsort(flat_e, kind="stable")
    counts = np.bincount(flat_e, minlength=E)
    offs = np.concatenate([[0], np.cumsum(counts)])

    C = int(np.ceil(min(counts.max(), CAPACITY) / P) * P)
    if C % 512 == P and DTYPE == "f32r":
        C += P  # avoid a 128-wide tail chunk (f32r slow below 256 free dim)

    # Fold the gate into x (g * relu(z) == relu(g * z) since g > 0) —
    # only exact when b1 == 0, which setup_inputs guarantees.
    fold_gate = not np.any(b1)

    toks = []
    in_maps = []
    overflow = []
    for e in range(E):
        sel = order[offs[e] : offs[e + 1]]
        te = flat_tok[sel]
        ge = flat_g[sel]
        if len(te) > C:
            overflow.append((e, te[C:], ge[C:]))
            te = te[:C]
            ge = ge[:C]
        toks.append(te)
        cnt = len(te)
        xg_T = np.zeros((D, C), dtype=np.float32)
        if fold_gate:
            xg_T[:, :cnt] = (xf[te] * ge[:, None]).T
        else:
            xg_T[:, :cnt] = xf[te].T
        if DTYPE == "bf16":
            import ml_dtypes

            bf = ml_dtypes.bfloat16
            m = {
                "xt": xg_T.astype(bf),
                "w1": np.ascontiguousarray(w1[e]).astype(bf),
                "b1": np.ascontiguousarray(b1[e].reshape(H // P, P).T),
                "w2": np.ascontiguousarray(w2[e]).astype(bf),
            }
        else:
            m = {
                "xt": xg_T,
                "w1": np.ascontiguousarray(w1[e]),
                "b1": np.ascontiguousarray(b1[e].reshape(H // P, P).T),
                "w2": np.ascontiguousarray(w2[e]),
            }
        if not fold_gate:
            g_arr = np.zeros((C, 1), dtype=np.float32)
            g_arr[:cnt, 0] = ge
            m["g"] = g_arr
        in_maps.append(m)
    return in_maps, toks, C, fold_gate, idx, gates, loss, overflow


def kernel(x, w_gate, w1, b1, w2, b2):
    x = np.asarray(x, dtype=np.float32)
    w_gate = np.asarray(w_gate, dtype=np.float32)
    w1 = np.asarray(w1, dtype=np.float32)
    b1 = np.asarray(b1, dtype=np.float32)
    w2 = np.asarray(w2, dtype=np.float32)
    b2 = np.asarray(b2, dtype=np.float32)

    in_maps, toks, C, fold_gate, idx, gates, loss, overflow = _prepare(
        x, w_gate, w1, b1, w2
    )

    run = _get_runner(C, fold_gate, DTYPE)
    outs = run(in_maps)

    y = np.zeros((N, D), dtype=np.float32)
    for e in range(E):
        cnt = len(toks[e])
        y[toks[e]] += outs[e]["y"][:cnt]

    # capacity-overflow tokens: exact f32 FFN on host (b2 excluded here,
    # it is added for all tokens via gates_dense @ b2 below)
    for e, te, ge in overflow:
        xo = x.reshape(N, D)[te]
        ho = np.maximum(xo @ w1[e] + b1[e], 0.0)
        y[te] += ge[:, None] * (ho @ w2[e])

    # b2 contribution: y += sum_e gates_dense[:, e] * b2[e]
    gates_dense = np.zeros((N, E), dtype=np.float32)
    gates_dense[np.arange(N)[:, None], idx] = gates
    y += gates_dense @ b2

    return y.reshape(B, L, D), np.float32(loss)
